# revision 4
# baseline (speedup 1.0000x reference)
"""GAT-style kernel for Trainium2, 8 NeuronCores.

Math (per head, d = nhid):
    h  = xf @ W.T + b                  (N, d)
    h1 = h / max(||h||_row, eps)       row L2 normalize
    e  = h1 @ h1.T                     (N, N)  -- never materialized
    att = e / ||e||_col                column L2 normalize
    out = act(att @ h1)

Collapse: with G = h1.T @ h1 (d x d),
    ||e||_col[j]^2 = h1_j.T G h1_j     (row-wise quadratic form)
    att @ h1 = h1 @ M,  M = h1.T @ (h1 / cn)   (d x d)
so the N x N attention matrix is never formed and the whole computation
is O(N d^2).

Mode: "rep" -- every core gets the full input and computes the full
output; no collectives.  On this stack an AllReduce costs far more than
the whole collapsed computation, so replication wins despite 8x
redundant compute.

v5 (calibrated against the fake_nrt stack):
  * all matmuls in bf16 (measured ~12x cheaper than f32 on this stack:
    f32 mm ~280-980ns, bf16 mm ~20-80ns)
  * mega elementwise/reduction ops (DVE ~0.52ns/elem bf16) instead of
    per-tile chains; DVE<->ACT hops are cheap (~0.2us), PE-involved
    producer->consumer edges are ~2.7us latency but pipeline when many
    are in flight
  * PE transposes + drains replaced by 3 XBAR DMA-transpose
    instructions (bf16, out[p, a, m] = in[m, a*128 + p], so the
    standard (128, t, d<=128) row-tile layout transposes directly into
    (d, t, 128) = (d, n))
  * PSUM drains batched, alternating DVE/ACT
"""

import sys

for _p in ("/opt/trn_rl_repo", "/root/.axon_site/_ro/trn_rl_repo"):
    if _p not in sys.path:
        sys.path.append(_p)

import numpy as np

N_CORES = 8
N = 4096
NLOC = N // N_CORES
NFEAT = 128
NHID = 64
NCLASS = 16
EPS = 1e-12

_prog_cache = {}
PHASES = {1, 2, 3, 4, 5, 6}  # contiguous-prefix gating for profiling


def _patch_tile_drain():
    """Walrus in this container rejects Tile's tail drain (too many sync
    waits on one instruction).  Split it into one-wait-per-drain."""
    import concourse.tile as tile
    from concourse.vector_clock import ScopedClock, VectorClock

    if getattr(tile.TileContext, "_drain_split_patched", False):
        return

    def _drain_and_barrier(self, tick_clock, wait_clock):
        nc = self.nc
        gvc = tick_clock.global_clock  # VectorClock
        n = len(gvc)
        for proc in range(n):
            t = gvc[proc]
            if t > 0:
                sub = VectorClock([t if i == proc else 0 for i in range(n)])
                d = nc.sync.drain()
                wait_clock.add_sem_waits(d.ins, ScopedClock({None: sub}))
        nc.all_engine_barrier()
        assert self.sems is not None
        popped = nc._tile_sem_poison_stack.pop()
        assert popped is self._sem_poison
        nc.clear_and_free_semaphores(list(self.sems.allocated().values()))
        nc.all_engine_barrier()

    tile.TileContext._drain_and_barrier = _drain_and_barrier
    tile.TileContext._drain_split_patched = True


def _split_multi_waits(nc):
    """This container's walrus allows only one sync-wait per instruction.
    Hoist extra waits onto standalone same-engine NoOps."""
    import concourse.mybir as mybir

    n_new = 0
    for blk in nc.main_func.blocks:
        out = []
        changed = False
        for inst in blk.instructions:
            si = inst.sync_info
            waits = list(si.on_wait) if (si and si.on_wait) else []
            if len(waits) > 1:
                changed = True
                for w in waits[:-1]:
                    nop = mybir.InstNoOp(name=f"{inst.name}-xw{n_new}", ins=[], outs=[])
                    n_new += 1
                    nop.engine = inst.engine
                    nop.sync_info = mybir.SyncInfo(on_wait=[w], on_update=[])
                    out.append(nop)
                si.on_wait = [waits[-1]]
                inst.sync_info = si
            out.append(inst)
        if changed:
            blk.instructions = out


def _emit_body(nc, tc, ctx, tensors, r, nloc, with_bias):
    import concourse.bass as bass
    import concourse.mybir as mybir
    from concourse.bass import ts

    f32 = mybir.dt.float32
    bf16 = mybir.dt.bfloat16
    nt = nloc // 128
    nch = nloc // 512
    D3 = 3 * NHID

    x_d = tensors["xloc"]
    w123t_d = tensors["w123t"]
    b123_d = tensors["b123"]
    wot_hi_d = tensors["wot_hi"]
    wot_lo_d = tensors["wot_lo"]
    bo_d = tensors["bo"]
    out_d = tensors["outt"]

    const = ctx.enter_context(tc.tile_pool(name=f"const{r}", bufs=1))
    work = ctx.enter_context(tc.tile_pool(name=f"work{r}", bufs=1))
    psum = ctx.enter_context(tc.tile_pool(name=f"psum{r}", bufs=1, space="PSUM"))

    def ps_tile(shape, tag, name, bufs=1):
        return psum.tile(shape, f32, tag=tag, name=f"{name}_{r}", bufs=bufs)

    def bcast_free(ap, inner):
        """Broadcast an AP with a trailing stride-0 inner dim."""
        return bass.AP(tensor=ap.tensor, offset=ap.offset, ap=[*ap.ap, [0, inner]])

    copy_flip = [0]

    def drain_copy(dst, src):
        """PSUM->SBUF copy, alternating DVE/ACT to split the sync load."""
        if copy_flip[0] & 1:
            nc.scalar.copy(dst, src)
        else:
            nc.vector.tensor_copy(dst, src)
        copy_flip[0] += 1

    # ---- constants / inputs (host-cast to bf16) ----
    w123t_bf = const.tile([128, D3], bf16, name=f"w123tb{r}")
    nc.sync.dma_start(out=w123t_bf[:], in_=w123t_d[:])
    wot_hi_bf = const.tile([128, NCLASS], bf16, name=f"wot_hib{r}")
    nc.sync.dma_start(out=wot_hi_bf[:], in_=wot_hi_d[:])
    wot_lo_bf = const.tile([64, NCLASS], bf16, name=f"wot_lob{r}")
    nc.sync.dma_start(out=wot_lo_bf[:], in_=wot_lo_d[:])
    if with_bias:
        b123_bf = const.tile([1, D3], bf16, name=f"b123b{r}")
        nc.sync.dma_start(out=b123_bf[:], in_=b123_d[:])
        bo_bf = const.tile([1, NCLASS], bf16, name=f"bob{r}")
        nc.sync.dma_start(out=bo_bf[:], in_=bo_d[:])
        ones_row = const.tile([1, 128], bf16, name=f"ones{r}")
        nc.vector.memset(ones_row[:], 1.0)

    xbf = const.tile([128, nloc], bf16, name=f"xbf{r}")
    nc.sync.dma_start(out=xbf[:], in_=x_d[:])

    # persistent bf16 tensors
    h1a01 = const.tile([128, nt, 128], bf16, name=f"h1a01_{r}")
    h1a2 = const.tile([128, nt, 128], bf16, name=f"h1a2_{r}")  # cols 64:128 pad
    h1t01 = const.tile([128, nt, 128], bf16, name=f"h1t01_{r}")
    h1t2p = const.tile([128, nt, 128], bf16, name=f"h1t2p_{r}")
    hc01 = const.tile([128, nloc], bf16, name=f"hc01_{r}")
    hc2 = const.tile([64, nloc], bf16, name=f"hc2_{r}")
    # pad zeroing (pool engine; off the critical path)
    nc.gpsimd.memset(h1a2[:, :, 64:128], 0.0)

    if 1 not in PHASES:
        return

    # =================== S1: h = x @ W.T (+b), row norms ===================
    for i in range(nt // 2):
        ha_ps = ps_tile([128, 2, D3], "wide", f"ha{i}", bufs=3)
        for j in range(2):
            t = 2 * i + j
            nc.tensor.matmul(
                ha_ps[:, j, :], xbf[:, ts(t, 128)], w123t_bf[:],
                start=True, stop=not with_bias,
            )
            if with_bias:
                nc.tensor.matmul(
                    ha_ps[:, j, :], ones_row[:], b123_bf[:],
                    start=False, stop=True,
                )
        drain_copy(h1a01[:, ts(i, 2), :], ha_ps[:, :, 0:128])
        drain_copy(h1a2[:, ts(i, 2), 0:64], ha_ps[:, :, 128:192])

    # row norms: q = rowsum(h^2) per (tile, head)
    sq01 = work.tile([128, nt, 128], bf16, tag="scr1", name=f"sq01_{r}")
    nc.vector.tensor_mul(sq01[:], h1a01[:], h1a01[:])
    sq2 = work.tile([128, nt, 64], bf16, tag="scr2", name=f"sq2_{r}")
    nc.scalar.activation(
        sq2[:], h1a2[:, :, 0:64], mybir.ActivationFunctionType.Square
    )
    q01 = const.tile([128, nt, 2], f32, name=f"q01_{r}")
    nc.vector.reduce_sum(
        q01[:],
        sq01[:].rearrange("p t (k d) -> p (t k) d", k=2),
        axis=mybir.AxisListType.X,
    )
    q2 = const.tile([128, nt, 1], f32, name=f"q2_{r}")
    nc.vector.reduce_sum(q2[:], sq2[:], axis=mybir.AxisListType.X)
    qi01 = const.tile([128, nt, 2], f32, name=f"qi01_{r}")
    nc.vector.reciprocal(qi01[:], q01[:])
    qi2 = const.tile([128, nt, 1], f32, name=f"qi2_{r}")
    nc.vector.reciprocal(qi2[:], q2[:])
    rn01 = const.tile([128, nt, 2], bf16, name=f"rn01_{r}")
    nc.scalar.sqrt(rn01[:], qi01[:])
    rn2 = const.tile([128, nt, 1], bf16, name=f"rn2_{r}")
    nc.scalar.sqrt(rn2[:], qi2[:])
    nc.vector.tensor_mul(
        h1a01[:].rearrange("p t (k d) -> p (t k) d", k=2),
        h1a01[:].rearrange("p t (k d) -> p (t k) d", k=2),
        bcast_free(rn01[:], NHID),
    )
    nc.vector.tensor_mul(
        h1a2[:, :, 0:64], h1a2[:, :, 0:64], bcast_free(rn2[:], NHID)
    )

    # transposes: (128, t, d) -> (d, t, 128) == (d, n)
    nc.sync.dma_start(
        out=h1t01[:],
        in_=h1a01[:].rearrange("p a b -> p (a b)"),
        transpose=True,
    )
    nc.scalar.dma_start(
        out=h1t2p[:],
        in_=h1a2[:].rearrange("p a b -> p (a b)"),
        transpose=True,
    )

    # Gram: G01 (128x128, diag blocks valid), G2 (64x64)
    g01_ps = ps_tile([128, 128], "acc01", "g01")
    g2_ps = ps_tile([NHID, NHID], "accsm", "g2")
    for t in range(nt):
        nc.tensor.matmul(
            g01_ps[:], h1a01[:, t, :], h1a01[:, t, :],
            start=(t == 0), stop=(t == nt - 1),
        )
        nc.tensor.matmul(
            g2_ps[:], h1a2[:, t, 0:64], h1a2[:, t, 0:64],
            start=(t == 0), stop=(t == nt - 1),
        )
    gblk = const.tile([128, 128], bf16, name=f"gblk{r}")
    nc.vector.memset(gblk[:], 0.0)
    nc.vector.tensor_copy(gblk[0:64, 0:64], g01_ps[0:64, 0:64])
    nc.scalar.copy(gblk[64:128, 64:128], g01_ps[64:128, 64:128])
    g2_sb = const.tile([NHID, NHID], bf16, name=f"g2sb{r}")
    nc.vector.tensor_copy(g2_sb[:], g2_ps[:])

    if 2 not in PHASES:
        return

    # =================== S2: ta = h1 @ G, p, h1s, M ===================
    ta_sb = work.tile([128, nt, D3], bf16, tag="big", name=f"ta_sb_{r}")
    for i in range(nt // 2):
        ta_ps = ps_tile([128, 2, D3], "wide", f"ta{i}", bufs=3)
        for j in range(2):
            t = 2 * i + j
            nc.tensor.matmul(
                ta_ps[:, j, 0:128], h1t01[:, t, :], gblk[:],
                start=True, stop=True,
            )
            nc.tensor.matmul(
                ta_ps[:, j, 128:192], h1t2p[0:64, t, :], g2_sb[:],
                start=True, stop=True,
            )
        drain_copy(ta_sb[:, ts(i, 2), :], ta_ps[:])

    prod01 = work.tile([128, nt, 128], bf16, tag="scr1", name=f"prod01_{r}")
    nc.vector.tensor_mul(prod01[:], ta_sb[:, :, 0:128], h1a01[:])
    prod2 = work.tile([128, nt, 64], bf16, tag="scr2", name=f"prod2_{r}")
    nc.vector.tensor_mul(prod2[:], ta_sb[:, :, 128:192], h1a2[:, :, 0:64])
    p01 = const.tile([128, nt, 2], f32, name=f"p01_{r}")
    nc.vector.reduce_sum(
        p01[:],
        prod01[:].rearrange("p t (k d) -> p (t k) d", k=2),
        axis=mybir.AxisListType.X,
    )
    p2 = const.tile([128, nt, 1], f32, name=f"p2_{r}")
    nc.vector.reduce_sum(p2[:], prod2[:], axis=mybir.AxisListType.X)
    pi01 = const.tile([128, nt, 2], f32, name=f"pi01_{r}")
    nc.vector.reciprocal(pi01[:], p01[:])
    pi2 = const.tile([128, nt, 1], f32, name=f"pi2_{r}")
    nc.vector.reciprocal(pi2[:], p2[:])
    icn01 = const.tile([128, nt, 2], bf16, name=f"icn01_{r}")
    nc.scalar.sqrt(icn01[:], pi01[:])
    icn2 = const.tile([128, nt, 1], bf16, name=f"icn2_{r}")
    nc.scalar.sqrt(icn2[:], pi2[:])
    h1s01 = const.tile([128, nt, 128], bf16, name=f"h1s01_{r}")
    nc.vector.tensor_mul(
        h1s01[:].rearrange("p t (k d) -> p (t k) d", k=2),
        h1a01[:].rearrange("p t (k d) -> p (t k) d", k=2),
        bcast_free(icn01[:], NHID),
    )
    h1s2 = const.tile([128, nt, 64], bf16, name=f"h1s2_{r}")
    nc.vector.tensor_mul(
        h1s2[:], h1a2[:, :, 0:64], bcast_free(icn2[:], NHID)
    )

    m01_ps = ps_tile([128, 128], "acc01", "m01")
    m2_ps = ps_tile([NHID, NHID], "accsm", "m2")
    for t in range(nt):
        nc.tensor.matmul(
            m01_ps[:], h1a01[:, t, :], h1s01[:, t, :],
            start=(t == 0), stop=(t == nt - 1),
        )
        nc.tensor.matmul(
            m2_ps[:], h1a2[:, t, 0:64], h1s2[:, t, :],
            start=(t == 0), stop=(t == nt - 1),
        )
    mblk = const.tile([128, 128], bf16, name=f"mblk{r}")
    nc.vector.memset(mblk[:], 0.0)
    nc.vector.tensor_copy(mblk[0:64, 0:64], m01_ps[0:64, 0:64])
    nc.scalar.copy(mblk[64:128, 64:128], m01_ps[64:128, 64:128])
    m2_sb = const.tile([NHID, NHID], bf16, name=f"m2sb{r}")
    nc.vector.tensor_copy(m2_sb[:], m2_ps[:])

    if 3 not in PHASES:
        return

    # =================== S3: z = h1 @ M (transposed), elu -> hc ===========
    z01_sb = work.tile([128, nloc], bf16, tag="scr1", name=f"z01_{r}")
    z2_sb = work.tile([64, nloc], bf16, tag="scr2", name=f"z2_{r}")
    for c in range(nch):
        z01_ps = ps_tile([128, 512], "wide", f"z01_{c}", bufs=3)
        nc.tensor.matmul(
            z01_ps[:], mblk[:], h1t01[:, ts(c, 4), :], start=True, stop=True
        )
        drain_copy(z01_sb[:, ts(c, 512)], z01_ps[:])
        z2_ps = ps_tile([NHID, 512], "wide2", f"z2_{c}", bufs=2)
        nc.tensor.matmul(
            z2_ps[:], m2_sb[:], h1t2p[0:64, ts(c, 4), :], start=True, stop=True
        )
        drain_copy(z2_sb[:, ts(c, 512)], z2_ps[:])
    # elu(z) = min(exp(z), 1) + (max(z,0) - 1), 3 mega-ops per stream
    for zsrc, dst, parts, nm in ((z01_sb, hc01, 128, "01"), (z2_sb, hc2, 64, "2")):
        e_max = work.tile([parts, nloc], bf16, tag=f"emax{nm}", name=f"emax{nm}_{r}")
        nc.vector.tensor_scalar(
            out=e_max[:], in0=zsrc[:], scalar1=0.0, scalar2=-1.0,
            op0=mybir.AluOpType.max, op1=mybir.AluOpType.add,
        )
        e_exp = work.tile([parts, nloc], bf16, tag=f"eexp{nm}", name=f"eexp{nm}_{r}")
        nc.scalar.activation(e_exp[:], zsrc[:], mybir.ActivationFunctionType.Exp)
        nc.vector.scalar_tensor_tensor(
            out=dst[:], in0=e_exp[:], scalar=1.0, in1=e_max[:],
            op0=mybir.AluOpType.min, op1=mybir.AluOpType.add,
        )

    if 4 not in PHASES:
        return

    # =================== S4: output head: ho, norms, go ===================
    ho_sb = const.tile([128, nt, NCLASS], bf16, name=f"ho_sb{r}")
    for i in range(nt // 8):
        ho_ps = ps_tile([128, 8, NCLASS], "wide2", f"ho{i}", bufs=2)
        for j in range(8):
            t = 8 * i + j
            nc.tensor.matmul(
                ho_ps[:, j, :], hc01[:, ts(t, 128)], wot_hi_bf[:],
                start=True, stop=False,
            )
            nc.tensor.matmul(
                ho_ps[:, j, :], hc2[:, ts(t, 128)], wot_lo_bf[:],
                start=False, stop=not with_bias,
            )
            if with_bias:
                nc.tensor.matmul(
                    ho_ps[:, j, :], ones_row[:], bo_bf[:],
                    start=False, stop=True,
                )
        drain_copy(ho_sb[:, ts(i, 8), :], ho_ps[:])

    sqo = work.tile([128, nt, NCLASS], bf16, tag="scr2", name=f"sqo_{r}")
    nc.scalar.activation(sqo[:], ho_sb[:], mybir.ActivationFunctionType.Square)
    qo = const.tile([128, nt, 1], f32, name=f"qo_{r}")
    nc.vector.reduce_sum(qo[:], sqo[:], axis=mybir.AxisListType.X)
    qoi = const.tile([128, nt, 1], f32, name=f"qoi_{r}")
    nc.vector.reciprocal(qoi[:], qo[:])
    rno = const.tile([128, nt, 1], bf16, name=f"rno_{r}")
    nc.scalar.sqrt(rno[:], qoi[:])
    # h1o, padded to 128 cols for the DMA transpose
    h1o = const.tile([128, nt, 128], bf16, name=f"h1o_{r}")
    nc.gpsimd.memset(h1o[:, :, NCLASS:128], 0.0)
    nc.vector.tensor_mul(
        h1o[:, :, 0:NCLASS], ho_sb[:], bcast_free(rno[:], NCLASS)
    )
    h1otp = const.tile([128, nt, 128], bf16, name=f"h1otp_{r}")
    nc.sync.dma_start(
        out=h1otp[:],
        in_=h1o[:].rearrange("p a b -> p (a b)"),
        transpose=True,
    )

    go_ps = ps_tile([NCLASS, NCLASS], "accsm", "go_ps")
    for t in range(nt):
        nc.tensor.matmul(
            go_ps[:], h1o[:, t, 0:NCLASS], h1o[:, t, 0:NCLASS],
            start=(t == 0), stop=(t == nt - 1),
        )
    go_sb = const.tile([NCLASS, NCLASS], bf16, name=f"go_sb{r}")
    nc.vector.tensor_copy(go_sb[:], go_ps[:])

    if 5 not in PHASES:
        return

    # =================== S5: tao, po, h1so, mo ===================
    tao_sb = const.tile([128, nt, NCLASS], bf16, name=f"tao_sb{r}")
    for i in range(nt // 8):
        tao_ps = ps_tile([128, 8, NCLASS], "wide2", f"tao{i}", bufs=2)
        for j in range(8):
            t = 8 * i + j
            nc.tensor.matmul(
                tao_ps[:, j, :], h1otp[0:NCLASS, t, :], go_sb[:],
                start=True, stop=True,
            )
        drain_copy(tao_sb[:, ts(i, 8), :], tao_ps[:])

    prodo = work.tile([128, nt, NCLASS], bf16, tag="scr2", name=f"prodo_{r}")
    nc.vector.tensor_mul(prodo[:], tao_sb[:], h1o[:, :, 0:NCLASS])
    po = const.tile([128, nt, 1], f32, name=f"po_{r}")
    nc.vector.reduce_sum(po[:], prodo[:], axis=mybir.AxisListType.X)
    poi = const.tile([128, nt, 1], f32, name=f"poi_{r}")
    nc.vector.reciprocal(poi[:], po[:])
    icno = const.tile([128, nt, 1], bf16, name=f"icno_{r}")
    nc.scalar.sqrt(icno[:], poi[:])
    h1so = const.tile([128, nt, NCLASS], bf16, name=f"h1so_{r}")
    nc.vector.tensor_mul(
        h1so[:], h1o[:, :, 0:NCLASS], bcast_free(icno[:], NCLASS)
    )

    mo_ps = ps_tile([NCLASS, NCLASS], "accsm", "mo_ps")
    for t in range(nt):
        nc.tensor.matmul(
            mo_ps[:], h1o[:, t, 0:NCLASS], h1so[:, t, :],
            start=(t == 0), stop=(t == nt - 1),
        )
    mo_sb = const.tile([NCLASS, NCLASS], bf16, name=f"mo_sb{r}")
    nc.vector.tensor_copy(mo_sb[:], mo_ps[:])

    if 6 not in PHASES:
        return

    # =================== S6: out = (h1o @ Mo).T = Mo @ h1o.T ==============
    fot_sb = const.tile([NCLASS, nloc], f32, name=f"fot_sb{r}")
    for c in range(nch):
        fot_ps = ps_tile([NCLASS, 512], "wide2", f"fot_{c}", bufs=2)
        nc.tensor.matmul(
            fot_ps[:], mo_sb[:], h1otp[0:NCLASS, ts(c, 4), :],
            start=True, stop=True,
        )
        drain_copy(fot_sb[:, ts(c, 512)], fot_ps[:])
    nc.sync.dma_start(out=out_d[:], in_=fot_sb[:])


def build_program(reps=1, mode="rep", with_bias=False, loop=1):
    """Build the Bass program (shared by kernel() and test timing).

    loop > 1 wraps the body in an on-device For_i (timing amplification)."""
    key = (reps, mode, with_bias, loop, tuple(sorted(PHASES)))
    if key in _prog_cache:
        return _prog_cache[key]
    assert mode == "rep", "only rep mode is supported"

    _patch_tile_drain()
    import concourse.bass as bass
    import concourse.tile as tile
    import concourse.mybir as mybir
    from contextlib import ExitStack

    nloc = N

    f32 = mybir.dt.float32
    bf16 = mybir.dt.bfloat16
    nc = bass.Bass(num_devices=N_CORES)
    tensors = {
        "xloc": nc.dram_tensor("xloc", [128, nloc], bf16, kind="ExternalInput"),
        "w123t": nc.dram_tensor("w123t", [128, 3 * NHID], bf16, kind="ExternalInput"),
        "b123": nc.dram_tensor("b123", [1, 3 * NHID], bf16, kind="ExternalInput"),
        "wot_hi": nc.dram_tensor("wot_hi", [128, NCLASS], bf16, kind="ExternalInput"),
        "wot_lo": nc.dram_tensor("wot_lo", [64, NCLASS], bf16, kind="ExternalInput"),
        "bo": nc.dram_tensor("bo", [1, NCLASS], bf16, kind="ExternalInput"),
        "outt": nc.dram_tensor("outt", [NCLASS, nloc], f32, kind="ExternalOutput"),
    }

    with tile.TileContext(nc) as tc:
        if loop > 1:
            with tc.For_i(0, loop, 1):
                for r in range(reps):
                    with ExitStack() as ctx:
                        _emit_body(nc, tc, ctx, tensors, r, nloc, with_bias)
        else:
            for r in range(reps):
                with ExitStack() as ctx:
                    _emit_body(nc, tc, ctx, tensors, r, nloc, with_bias)

    _split_multi_waits(nc)
    _prog_cache[key] = nc
    return nc


def make_in_maps(x, W1, b1, W2, b2, W3, b3, Wo, bo, mode="rep"):
    import ml_dtypes

    bf = np.dtype(ml_dtypes.bfloat16)
    x_mem = np.asarray(x, dtype=np.float32).reshape(NFEAT, N).astype(bf)
    w123t = np.ascontiguousarray(
        np.concatenate(
            [np.asarray(W1).T, np.asarray(W2).T, np.asarray(W3).T], axis=1
        ).astype(np.float32)
    ).astype(bf)
    b123 = (
        np.concatenate([np.asarray(b1), np.asarray(b2), np.asarray(b3)])
        .reshape(1, 3 * NHID)
        .astype(np.float32)
        .astype(bf)
    )
    wot = np.ascontiguousarray(np.asarray(Wo).T.astype(np.float32))  # (192, 16)
    wot_hi = np.ascontiguousarray(wot[:128]).astype(bf)
    wot_lo = np.ascontiguousarray(wot[128:]).astype(bf)
    bo_r = np.asarray(bo).reshape(1, NCLASS).astype(np.float32).astype(bf)
    common = {
        "w123t": w123t,
        "b123": b123,
        "wot_hi": wot_hi,
        "wot_lo": wot_lo,
        "bo": bo_r,
    }
    return [{"xloc": x_mem, **common} for _ in range(N_CORES)]


def assemble_output(results, mode="rep"):
    full = results[0]["outt"]
    return np.ascontiguousarray(full.reshape(1, NCLASS, 64, 64), dtype=np.float32)


def kernel(x, W1, b1, W2, b2, W3, b3, Wo, bo):
    from concourse.bass_utils import run_bass_kernel_spmd

    with_bias = any(np.any(np.asarray(b)) for b in (b1, b2, b3, bo))
    nc = build_program(reps=1, mode="rep", with_bias=with_bias)
    in_maps = make_in_maps(x, W1, b1, W2, b2, W3, b3, Wo, bo, mode="rep")
    res = run_bass_kernel_spmd(nc, in_maps, list(range(N_CORES)))
    return assemble_output(res.results, mode="rep")
